# revision 26
# baseline (speedup 1.0000x reference)
"""EntityAttentionLayer on 8 Trainium2 NeuronCores.

Data-parallel over batch (16 batches/core). All matmuls bf16 with f32 PSUM
accumulation. Layouts are chosen so no transpose of activations is needed
on the PE: q/k are produced pre-transposed by the projection layout, v
naturally, and the attention output is transposed by the DMA xbar on
otherwise-idle DMA engines.

The PE instruction stream is software-pipelined three batches deep: batch
b's attention (logits -> softmax-numerator -> attnV) is interleaved
chunk-by-chunk with batch b+1's k/v projection chains and batch b-1's
output projection, so the in-order PE queue always has independent work
queued behind each attnV group while the Scalar/Vector engines compute
exp/mask for the current head pair. The final batch normalizes and
transposes per head-group half so its tail chain is mostly hidden under
its own attention.

Math note: the reference computes
    w = softmax(logits masked with -inf); w[nan] = 0
    w = w * diff; w = w / (sum(w) + 1e-8)
which equals
    num = exp(logits) * valid * diff
    w   = num / (sum(num) + 1e-8 * sum(exp(logits) * valid))
Folding the 1e-8 into the mask: M = valid * (diff + 1e-8) gives
    w ~= exp(logits) * M / sum(exp(logits) * M)
with an O(1e-8) absolute perturbation on w (negligible vs bf16 rounding).
Fully-masked rows: numerator is exactly 0 and the denominator gets +1e-25,
so those rows come out exactly 0, matching the reference's NaN->0 path.
"""

import numpy as np
import ml_dtypes

BS, NE, NQ = 128, 512, 128
DIN, EMB, ODIM = 512, 512, 512
H, HD = 8, 64
NCORES = 8
BPC = BS // NCORES          # batches per core
GRP = 4                     # batches per q-projection group
EC = DIN // 128             # contraction chunks (4)
BF16 = ml_dtypes.bfloat16


def _build_nc():
    import concourse.bacc as bacc
    import concourse.mybir as mybir
    import concourse.tile as tile

    f32 = mybir.dt.float32
    bf16 = mybir.dt.bfloat16

    nc = bacc.Bacc("TRN2", target_bir_lowering=False, debug=False,
                   num_devices=NCORES)

    ents_d = nc.dram_tensor("entsT", [BPC, DIN, NE], bf16, kind="ExternalInput")
    mask_d = nc.dram_tensor("maskT", [BPC, NE, NQ], bf16, kind="ExternalInput")
    win_d = nc.dram_tensor("w_inT", [DIN, 3 * EMB], bf16, kind="ExternalInput")
    wout_d = nc.dram_tensor("w_outT", [EMB, ODIM], bf16, kind="ExternalInput")
    pm_d = nc.dram_tensor("pmT", [NQ, BPC], f32, kind="ExternalInput")
    out_d = nc.dram_tensor("out", [BPC, NQ, ODIM], f32, kind="ExternalOutput")

    NGRP = BPC // GRP

    with tile.TileContext(nc) as tc:
        with (
            tc.tile_pool(name="const", bufs=1) as cpool,
            tc.tile_pool(name="gwork", bufs=2) as gwork,
            tc.tile_pool(name="work", bufs=3) as work,
            tc.tile_pool(name="nums", bufs=6) as nums,
            tc.tile_pool(name="ps", bufs=5, space="PSUM") as ps,
            tc.tile_pool(name="ps_att", bufs=2, space="PSUM") as ps_att,
            tc.tile_pool(name="ps_out", bufs=1, space="PSUM") as ps_out,
        ):
            # ---- constants. The warm-up tile is memset first (no other
            # dependency) so PE warm-up matmuls start right after the Tile
            # preamble. DMA-fabric bandwidth is the startup bottleneck, so
            # every startup transfer rides ONE queue (sync) in priority
            # order: ents-b0/K-block chunk pairs (the critical path for the
            # first k-proj matmuls), then ents b1, V block, ents b2, Q
            # block, ents b3, w_out. ----
            warm = cpool.tile([128, 128], bf16)
            nc.gpsimd.memset(warm, 0.0)
            pm_sb = cpool.tile([128, BPC], f32)
            nc.gpsimd.dma_start(out=pm_sb, in_=pm_d.ap())

            w_in_sb = cpool.tile([128, EC, 3 * EMB], bf16)
            win_r = win_d.ap().rearrange("(c p) f -> p c f", p=128)
            w_out_sb = cpool.tile([128, EC, ODIM], bf16)
            ents0_sb = gwork.tile([128, EC, GRP, NE], bf16, name="ents_sb")
            ents0_src = [
                ents_d.ap()[i].rearrange("(c p) n -> p c n", p=128)
                for i in range(GRP)]
            for ce in range(EC):
                nc.sync.dma_start(out=ents0_sb[:, ce, 0, :],
                                  in_=ents0_src[0][:, ce, :])
                nc.sync.dma_start(out=w_in_sb[:, ce, EMB:2 * EMB],
                                  in_=win_r[:, ce, EMB:2 * EMB])
            nc.sync.dma_start(out=w_in_sb[:, :, 2 * EMB:3 * EMB],
                              in_=win_r[:, :, 2 * EMB:3 * EMB])
            nc.sync.dma_start(out=ents0_sb[:, :, 1, :], in_=ents0_src[1])
            nc.sync.dma_start(out=w_in_sb[:, :, 0:EMB],
                              in_=win_r[:, :, 0:EMB])
            nc.sync.dma_start(out=ents0_sb[:, :, 2, :], in_=ents0_src[2])
            nc.sync.dma_start(out=ents0_sb[:, :, 3, :], in_=ents0_src[3])
            nc.sync.dma_start(
                out=w_out_sb,
                in_=wout_d.ap().rearrange("(c p) f -> p c f", p=128))

            # warm-up matmuls: keep the PE HAM busy while the first weight
            # and entity DMAs are in flight so real matmuls start at 2.4 GHz
            psum_w = ps.tile([128, 128], f32, tag="big", name="psum_w")
            for _ in range(18):
                nc.tensor.matmul(psum_w, lhsT=warm, rhs=warm,
                                 start=True, stop=True)
            # identity for the final batch's PE-transposed chunks — emitted
            # after the warm-ups so it doesn't gate them
            ident = cpool.tile([128, 128], bf16)
            from concourse.masks import make_identity
            make_identity(nc, ident)

            # ---------------- per-batch pieces ----------------

            def kv_alloc(b):
                """allocate k/v tiles + issue the mask DMA for batch b."""
                mask_sb = work.tile([128, EC, NQ], bf16, name="mask_sb")
                nc.gpsimd.dma_start(
                    out=mask_sb,
                    in_=mask_d.ap()[b].rearrange("(c p) q -> p c q", p=128))
                kT_sb = work.tile([128, 4, NE], bf16, name="kT_sb")
                v_sb = work.tile([128, 4, H, HD + 1], bf16, name="v_sb")
                nc.gpsimd.memset(v_sb[:, :, :, HD], 1.0)
                return mask_sb, kT_sb, v_sb

            def kv_chain(kv, ents_sb, i, c):
                """one k-proj (c<4) or v-proj (c>=4) accumulation chain."""
                _, kT_sb, v_sb = kv
                if c < 4:
                    cf = c
                    psum_k = ps.tile([128, NE], f32, tag="big", name="psum_k")
                    for ce in range(EC):
                        nc.tensor.matmul(
                            psum_k,
                            lhsT=w_in_sb[:, ce,
                                         EMB + 128 * cf:EMB + 128 * (cf + 1)],
                            rhs=ents_sb[:, ce, i, :],
                            start=(ce == 0), stop=(ce == EC - 1))
                    if cf % 2 == 0:
                        nc.scalar.copy(kT_sb[:, cf, :], psum_k)
                    else:
                        nc.vector.tensor_copy(kT_sb[:, cf, :], psum_k)
                else:
                    cn = c - 4
                    psum_v = ps.tile([128, EMB], f32, tag="big", name="psum_v")
                    for ce in range(EC):
                        nc.tensor.matmul(
                            psum_v,
                            lhsT=ents_sb[:, ce, i, 128 * cn:128 * (cn + 1)],
                            rhs=w_in_sb[:, ce, 2 * EMB:3 * EMB],
                            start=(ce == 0), stop=(ce == EC - 1))
                    src = psum_v.rearrange("p (h d) -> p h d", h=H)
                    if cn % 2 == 0:
                        nc.scalar.copy(v_sb[:, cn, :, 0:HD], src)
                    else:
                        nc.vector.tensor_copy(v_sb[:, cn, :, 0:HD], src)

            def qproj(ents_sb):
                """fused qT projection for a whole group of 4 batches."""
                qT_sb = gwork.tile([128, 4, GRP, NQ], bf16, name="qT_sb")
                for cf in range(4):
                    psum_q = ps.tile([128, GRP, NQ], f32, tag="big",
                                     name="psum_q")
                    for ce in range(EC):
                        nc.tensor.matmul(
                            psum_q,
                            lhsT=w_in_sb[:, ce, 128 * cf:128 * (cf + 1)],
                            rhs=ents_sb[:, ce, :, 0:NQ],
                            start=(ce == 0), stop=(ce == EC - 1))
                    if cf % 2 == 0:
                        nc.scalar.copy(qT_sb[:, cf, :, :], psum_q)
                    else:
                        nc.vector.tensor_copy(qT_sb[:, cf, :, :], psum_q)
                return qT_sb

            def qproj_half(qT_sb, ents_sb, lo):
                """qT projection for 2 batches — used for group 0 so the
                first half doesn't wait on the later entity slab DMAs."""
                for cf in range(4):
                    psum_q = ps.tile([128, 2, NQ], f32, tag="big",
                                     name="psum_qh")
                    for ce in range(EC):
                        nc.tensor.matmul(
                            psum_q,
                            lhsT=w_in_sb[:, ce, 128 * cf:128 * (cf + 1)],
                            rhs=ents_sb[:, ce, lo:lo + 2, 0:NQ],
                            start=(ce == 0), stop=(ce == EC - 1))
                    if cf % 2 == 0:
                        nc.scalar.copy(qT_sb[:, cf, lo:lo + 2, :], psum_q)
                    else:
                        nc.vector.tensor_copy(qT_sb[:, cf, lo:lo + 2, :],
                                              psum_q)

            def logits(qT_sb, i, kT_sb, hc):
                """head-pair (2hc, 2hc+1) logits: dual row-group matmuls."""
                psl = [
                    ps.tile([128, 4, NQ], f32, tag="big", name="psl0"),
                    ps.tile([128, 4, NQ], f32, tag="big", name="psl1"),
                ]
                for cn in range(4):
                    for r in range(2):
                        nc.tensor.matmul(
                            psl[r][:, cn, :],
                            lhsT=kT_sb[64 * r:64 * (r + 1), hc,
                                       128 * cn:128 * (cn + 1)],
                            rhs=qT_sb[64 * r:64 * (r + 1), hc, i, :],
                            start=True, stop=True)
                return psl

            def attn_v(mask_sb, v_sb, hc, psl, att_tiles):
                """exp -> mask-multiply -> attnV for head pair (2hc, 2hc+1)."""
                for r in range(2):
                    h = 2 * hc + r
                    exp_sb = nums.tile([128, 4, NQ], bf16, tag="exp",
                                       name="exp_sb")
                    nc.scalar.activation(
                        exp_sb, psl[r],
                        mybir.ActivationFunctionType.Exp, scale=1.0 / 8.0)
                    num_sb = nums.tile([128, 4, NQ], bf16, tag="num",
                                       name="num_sb")
                    nc.vector.tensor_mul(num_sb, exp_sb, mask_sb)
                    patt, j = att_tiles[h // 4], h % 4
                    for cn in range(4):
                        nc.tensor.matmul(
                            patt[:, j, :],
                            lhsT=num_sb[:, cn, :],
                            rhs=v_sb[:, cn, h, :],
                            start=(cn == 0), stop=(cn == 3))

            def norm_half(att_tiles, t, deps_sb, recip_sb, attn_sb):
                """denominator + normalize for heads 4t..4t+3 (Vector)."""
                nc.vector.tensor_scalar_add(
                    deps_sb[:, 4 * t:4 * (t + 1)],
                    att_tiles[t][:, :, HD], 1e-25)
                nc.vector.reciprocal(recip_sb[:, 4 * t:4 * (t + 1)],
                                     deps_sb[:, 4 * t:4 * (t + 1)])
                nc.vector.tensor_mul(
                    attn_sb[:, 4 * t:4 * (t + 1), :],
                    att_tiles[t][:, :, 0:HD],
                    recip_sb[:, 4 * t:4 * (t + 1), None]
                    .broadcast_to([128, 4, HD]))

            def out_proj_mms(psum_o, attnT_sb, cts):
                for ct in cts:
                    nc.tensor.matmul(
                        psum_o,
                        lhsT=attnT_sb[:, ct, :],
                        rhs=w_out_sb[:, ct, :],
                        start=(ct == 0), stop=(ct == 3))

            # ---------------- main pipeline ----------------
            ents_tiles = [ents0_sb] + [None] * (NGRP - 1)
            kv_state = [None] * (BPC + 1)       # (mask, kT, v) per batch
            attn_prev = [None]                  # normalized attn of b-1
            attnT_prev = [None]
            psumo_prev = [None]
            qT_cur = [None]

            # pipeline fill: batch 0's k/v chains run standalone
            kv_state[0] = kv_alloc(0)
            for c in range(8):
                kv_chain(kv_state[0], ents0_sb, 0, c)

            for b in range(BPC):
                g, i = divmod(b, GRP)
                last = (b == BPC - 1)
                # the last batch normalizes + transposes inline (per
                # head-group half, during its own attention) so nothing of
                # its tail is left fully exposed at the end
                inline = last

                if b >= 1:
                    psumo_prev[0] = ps_out.tile([128, ODIM], f32, tag="out",
                                                name="psum_o")
                if g + 1 < NGRP:
                    if i == 0:
                        ents_tiles[g + 1] = gwork.tile(
                            [128, EC, GRP, NE], bf16, name="ents_sb")
                    nc.sync.dma_start(
                        out=ents_tiles[g + 1][:, :, i, :],
                        in_=ents_d.ap()[(g + 1) * GRP + i]
                            .rearrange("(c p) n -> p c n", p=128))

                if i == 0:
                    if b == 0:
                        # group 0: first half only (batches 0-1); the
                        # second half is emitted mid-attention once the
                        # later entity slabs have landed
                        qT_cur[0] = gwork.tile([128, 4, GRP, NQ], bf16,
                                               name="qT_sb")
                        qproj_half(qT_cur[0], ents_tiles[0], 0)
                    else:
                        qT_cur[0] = qproj(ents_tiles[g])

                if not last:
                    nb = b + 1
                    ng, ni = divmod(nb, GRP)
                    kv_state[nb] = kv_alloc(nb)

                mask_sb, kT_sb, v_sb = kv_state[b]
                att_tiles = [
                    ps_att.tile([128, 4, HD + 1], f32, tag="att", name="pa0"),
                    ps_att.tile([128, 4, HD + 1], f32, tag="att", name="pa1"),
                ]
                deps_sb = work.tile([128, H], f32, name="deps_sb")
                recip_sb = work.tile([128, H], f32, name="recip_sb")
                attn_sb = work.tile([128, H, HD], bf16, name="attn_sb")
                if inline:
                    attnT_il = work.tile([128, 4, 128], bf16,
                                         name="attnT_sb")
                af = attn_sb.rearrange("p h d -> p (h d)")

                def fillers(n):
                    # short-lived scratch tile per burst: its release (the
                    # burst's own last write) never waits on later queue
                    # entries, so pool rotation stays deadlock-free
                    pwf = ps.tile([128, 128], f32, tag="big", name="pwf")
                    for _ in range(n):
                        nc.tensor.matmul(pwf, lhsT=warm, rhs=warm,
                                         start=True, stop=True)
                if last:
                    out_lsb = work.tile([128, ODIM], f32, name="out_lsb")

                for hc in range(4):
                    psl = logits(qT_cur[0], i, kT_sb, hc)
                    # independent PE work to cover the exp/num latency of
                    # this head pair: next batch's kv chain, b-1's output
                    # projection, or warm filler matmuls on the last batch
                    if not last:
                        kv_chain(kv_state[b + 1], ents_tiles[ng], ni, hc)
                        kv_chain(kv_state[b + 1], ents_tiles[ng], ni, hc + 4)
                        if hc == 2 and b == 0:
                            qproj_half(qT_cur[0], ents_tiles[0], 2)
                        if hc == 2 and b >= 1:
                            out_proj_mms(psumo_prev[0], attnT_prev[0], (0, 1))
                        if hc == 3 and b >= 1:
                            out_proj_mms(psumo_prev[0], attnT_prev[0], (2, 3))
                            out_sb = work.tile([128, ODIM], f32,
                                               name="out_sb")
                            nc.scalar.activation(
                                out_sb, psumo_prev[0],
                                mybir.ActivationFunctionType.Copy,
                                scale=pm_sb[:, b - 1:b])
                            nc.sync.dma_start(out=out_d.ap()[b - 1],
                                              in_=out_sb)
                    else:
                        # batch 14's transposes completed during batch 14,
                        # so its out-projection runs in the early slots
                        if hc == 0:
                            out_proj_mms(psumo_prev[0], attnT_prev[0], (0, 1))
                            fillers(14)
                        if hc == 1:
                            out_proj_mms(psumo_prev[0], attnT_prev[0], (2, 3))
                            fillers(14)
                        if hc == 2:
                            out_sb = work.tile([128, ODIM], f32,
                                               name="out_sb")
                            nc.scalar.activation(
                                out_sb, psumo_prev[0],
                                mybir.ActivationFunctionType.Copy,
                                scale=pm_sb[:, b - 1:b])
                            nc.sync.dma_start(out=out_d.ap()[b - 1],
                                              in_=out_sb)
                            fillers(18)
                        if hc == 3:
                            fillers(18)
                    attn_v(mask_sb, v_sb, hc, psl, att_tiles)
                    if inline and hc == 2:
                        # heads 0-3 done: normalize + transpose them while
                        # heads 4-7 compute
                        norm_half(att_tiles, 0, deps_sb, recip_sb, attn_sb)
                        for ct in range(2):
                            nc.sync.dma_start_transpose(
                                attnT_il[:, ct, :],
                                af[:, 128 * ct:128 * (ct + 1)])

                if not inline:
                    # normalize, then transpose immediately on the DMA xbar
                    # (same sync-queue order as deferring to the next batch,
                    # but keeps the emission local)
                    norm_half(att_tiles, 0, deps_sb, recip_sb, attn_sb)
                    norm_half(att_tiles, 1, deps_sb, recip_sb, attn_sb)
                    attnT_sb = work.tile([128, 4, 128], bf16,
                                         name="attnT_sb")
                    for ct in range(4):
                        nc.sync.dma_start_transpose(
                            attnT_sb[:, ct, :],
                            af[:, 128 * ct:128 * (ct + 1)])
                    attnT_prev[0] = attnT_sb
                else:
                    # batch 15 tail: PE-transpose the second half (the DMA
                    # xbar path would be fully exposed here), then the
                    # output projection in two output-column halves so the
                    # first store overlaps the second half's matmuls
                    norm_half(att_tiles, 1, deps_sb, recip_sb, attn_sb)
                    for ct in range(2, 4):
                        pt = ps.tile([128, 128], bf16, tag="big", name="pt")
                        nc.tensor.transpose(
                            pt, af[:, 128 * ct:128 * (ct + 1)], ident)
                        if ct % 2 == 0:
                            nc.scalar.copy(attnT_il[:, ct, :], pt)
                        else:
                            nc.vector.tensor_copy(attnT_il[:, ct, :], pt)
                    fillers(8)
                    psum_oh = [
                        ps.tile([128, 256], f32, tag="big", name="poh0"),
                        ps.tile([128, 256], f32, tag="big", name="poh1"),
                    ]
                    for half in range(2):
                        sl = slice(256 * half, 256 * (half + 1))
                        for ct in range(4):
                            nc.tensor.matmul(
                                psum_oh[half],
                                lhsT=attnT_il[:, ct, :],
                                rhs=w_out_sb[:, ct, sl],
                                start=(ct == 0), stop=(ct == 3))
                        nc.vector.tensor_scalar_mul(
                            out_lsb[:, sl], psum_oh[half],
                            pm_sb[:, b:b + 1])
                        nc.sync.dma_start(out=out_d.ap()[b][:, sl],
                                          in_=out_lsb[:, sl])

    nc.compile()
    return nc


def _prep_inputs(entities, pre_mask, diff_mask, post_mask, W_in, W_out):
    entities = np.asarray(entities, dtype=np.float32)
    pre_mask = np.asarray(pre_mask, dtype=bool)
    diff_mask = np.asarray(diff_mask, dtype=np.float32)
    post_mask = np.asarray(post_mask, dtype=bool)
    W_in = np.asarray(W_in, dtype=np.float32)
    W_out = np.asarray(W_out, dtype=np.float32)

    entsT = np.ascontiguousarray(entities.transpose(0, 2, 1)).astype(BF16)
    m = (~pre_mask).astype(np.float32) * (diff_mask + 1e-8)
    maskT = np.ascontiguousarray(m.transpose(0, 2, 1)).astype(BF16)
    w_inT = np.ascontiguousarray(W_in.T).astype(BF16)
    w_outT = np.ascontiguousarray(W_out.T).astype(BF16)
    pmT = np.ascontiguousarray((~post_mask).T.astype(np.float32))

    in_maps = []
    for c in range(NCORES):
        sl = slice(c * BPC, (c + 1) * BPC)
        in_maps.append({
            "entsT": np.ascontiguousarray(entsT[sl]),
            "maskT": np.ascontiguousarray(maskT[sl]),
            "w_inT": w_inT,
            "w_outT": w_outT,
            "pmT": np.ascontiguousarray(pmT[:, sl]),
        })
    return in_maps


def _run(in_maps, trace=False):
    from concourse.bass_utils import run_bass_kernel_spmd
    nc = _build_nc()
    last_exc = None
    for attempt in range(3):
        try:
            return run_bass_kernel_spmd(
                nc, in_maps, core_ids=list(range(NCORES)), trace=trace)
        except Exception as e:  # transient NRT_EXEC_UNIT faults on fresh NEFFs
            last_exc = e
            import time
            time.sleep(2.0 * (attempt + 1))
    raise last_exc


def kernel_traced(entities, pre_mask, diff_mask, post_mask, W_in, W_out, b_out,
                  trace=False):
    """Returns (output, BassKernelResults)."""
    b_out = np.asarray(b_out, dtype=np.float32)
    post_mask_np = np.asarray(post_mask, dtype=bool)
    in_maps = _prep_inputs(entities, pre_mask, diff_mask, post_mask, W_in, W_out)
    res = _run(in_maps, trace=trace)
    out = np.concatenate([r["out"] for r in res.results], axis=0)
    # faithfulness: reference adds b_out before the post-mask zeroing
    out = out + np.where(post_mask_np[:, :, None], 0.0, b_out[None, None, :])
    return out.astype(np.float32), res


def kernel(entities, pre_mask, diff_mask, post_mask, W_in, W_out, b_out):
    out, _ = kernel_traced(entities, pre_mask, diff_mask, post_mask,
                           W_in, W_out, b_out)
    return out
